# revision 11
# baseline (speedup 1.0000x reference)
"""Guide-token attention kernel for Trainium2 (8 NeuronCores).

Module: y[b] = softmax(((Q+tQ) @ (K+tK)^T)/sqrt(hd)) @ V  per head, where
  Q = x @ Wq^T + bq, K = x @ Wk^T + bk, V = x @ Wv^T + bv,
  tQ/tK are projections of a per-batch guide token (broadcast over seq).

Shapes: x [4, 1024, 1024], tokens [4, 1, 1024], W* [1024, 1024], b* [1024].
H=16 heads, hd=64.

Sharding: 8 cores = 4 batches x 2 head-groups (8 heads each); weights
column-sharded per head group; each core sees one batch -> no cross-core
communication.

Design notes (v2 — schedule rework of the 149us baseline):
  - PE is the bottleneck: ~384 N=512 matmul slots x ~220 ns ≈ 86 us
    (192 proj + 64 row-tiled score pairs + 128 AV).  Everything else
    (ACT exp 71 us, DVE ~40 us) must hide under the PE stream.
  - Input DMA split across BOTH hardware DGE queues (sync + scalar)
    with chunked SBUF tiles so the first projection matmuls start ~3 us
    in, and x/weight chunks unblock matmuls incrementally.
  - One unified PE program: per score pair (4 MMs, 2 exps) we interleave
    ~5 projection-filler MMs and up to 2 trailing AV groups, keeping PE
    dense and rate-matched against ACT while AV trails scores by >=2
    pairs (probs guaranteed ready -> no in-order stalls).
  - Normalize: reciprocal straight off the PSUM denominator row (no den
    copy), GpSimd partition-broadcast, one DVE multiply -> bf16 yT.
  - Output DMAd per 128-feature tile as soon as its 4 normalizes land.
"""

import os

import numpy as np
import ml_dtypes

import concourse.bass as bass
import concourse.tile as tile
from concourse import bacc
from concourse import mybir
from concourse.bass_utils import run_bass_kernel_spmd

B = 4
S = 1024
D = 1024
H = 16
HD = 64
NCORES = 8
FPG = 512          # features per head-group (8 heads * 64)
NKC = D // 128     # contraction chunks for projections
NFT = FPG // 128   # feature tiles per group
NST = S // 128     # sequence tiles
NQB = S // 512     # 512-wide query blocks
HPG = 8            # heads per group
NPAIR = NST // 2   # kt pairs per unit

BF16 = mybir.dt.bfloat16
F32 = mybir.dt.float32

# x chunk arrival order (see DMA queue layout below)
KC_ORDER = [0, 1, 4, 5, 2, 3, 6, 7]
FILLERS_PER_PAIR = 5

_CACHE = {}


def _build():
    nc = bacc.Bacc()

    # ---- DRAM inputs, chunked for DMA-queue parallelism + early unblock ----
    # sync queue:   x01, x23, wkA, qadd, kadd, wv   (+ output yT later)
    # scalar queue: wqA, x45, x67, wqB, wkB
    xd = {}
    for cname in ("xA0", "xB0", "xC0", "xD0", "xA1", "xB1", "xC1", "xD1"):
        xd[cname] = nc.declare_dram_parameter(cname, [128, 2, 512], BF16, isOutput=False)
    wqA = nc.declare_dram_parameter("wqA", [128, NKC, 256], BF16, isOutput=False)
    wqB = nc.declare_dram_parameter("wqB", [128, NKC, 256], BF16, isOutput=False)
    wkA = nc.declare_dram_parameter("wkA", [128, NKC, 256], BF16, isOutput=False)
    wkB = nc.declare_dram_parameter("wkB", [128, NKC, 256], BF16, isOutput=False)
    wvd = nc.declare_dram_parameter("wv", [128, NKC, FPG], BF16, isOutput=False)
    qaddd = nc.declare_dram_parameter("qadd", [128, NFT], F32, isOutput=False)
    kaddd = nc.declare_dram_parameter("kadd", [128, NFT], F32, isOutput=False)
    yTd = nc.declare_dram_parameter("yT", [NFT, 128, S], BF16, isOutput=True)

    with tile.TileContext(nc) as tc:
        with (
            tc.tile_pool(name="persist", bufs=1) as persist,
            tc.tile_pool(name="probs", bufs=36) as probs_pool,
            tc.tile_pool(name="norm", bufs=4) as norm_pool,
            tc.tile_pool(name="psP", bufs=2, space=bass.MemorySpace.PSUM) as psP,
            tc.tile_pool(name="psA", bufs=2, space=bass.MemorySpace.PSUM) as psA,
            tc.tile_pool(name="psAV", bufs=2, space=bass.MemorySpace.PSUM) as psAV,
        ):
            # ---- persistent SBUF tensors (chunked to match DMA granularity) ----
            xts = {(c, h): persist.tile([128, 2, 512], BF16, name=f"xt{c}{h}")
                   for c in range(4) for h in range(2)}
            wq_sb = {"A": persist.tile([128, NKC, 256], BF16, name="wq_a"),
                     "B": persist.tile([128, NKC, 256], BF16, name="wq_b")}
            wk_sb = {"A": persist.tile([128, NKC, 256], BF16, name="wk_a"),
                     "B": persist.tile([128, NKC, 256], BF16, name="wk_b")}
            wv_sb = persist.tile([128, NKC, FPG], BF16)
            qa = persist.tile([128, NFT], F32)
            ka = persist.tile([128, NFT], F32)
            cq = [persist.tile([128, S], BF16, name=f"cq{i}") for i in range(NFT)]   # cQT/8
            ck = [persist.tile([128, S], BF16, name=f"ck{i}") for i in range(NFT)]   # cKT
            vts = [persist.tile([128, HPG, HD + 1], BF16, name=f"vt{i}") for i in range(NST)]
            yt = persist.tile([128, NFT, S], BF16)
            wrm = persist.tile([128, 512], BF16)

            # ---- input DMAs on both HW DGE queues ----
            nc.sync.dma_start(out=xts[(0, 0)][:], in_=xd["xA0"][:])
            nc.sync.dma_start(out=xts[(1, 0)][:], in_=xd["xB0"][:])
            nc.sync.dma_start(out=qa[:], in_=qaddd[:])
            nc.sync.dma_start(out=ka[:], in_=kaddd[:])
            nc.sync.dma_start(out=wk_sb["A"][:], in_=wkA[:])
            nc.sync.dma_start(out=xts[(0, 1)][:], in_=xd["xA1"][:])
            nc.sync.dma_start(out=xts[(1, 1)][:], in_=xd["xB1"][:])
            nc.sync.dma_start(out=wv_sb[:], in_=wvd[:])
            nc.scalar.dma_start(out=wq_sb["A"][:], in_=wqA[:])
            nc.scalar.dma_start(out=xts[(2, 0)][:], in_=xd["xC0"][:])
            nc.scalar.dma_start(out=xts[(3, 0)][:], in_=xd["xD0"][:])
            nc.scalar.dma_start(out=xts[(2, 1)][:], in_=xd["xC1"][:])
            nc.scalar.dma_start(out=xts[(3, 1)][:], in_=xd["xD1"][:])
            nc.scalar.dma_start(out=wq_sb["B"][:], in_=wqB[:])
            nc.scalar.dma_start(out=wk_sb["B"][:], in_=wkB[:])

            # ones columns for the AV denominator rows + HAM warmup source
            nc.vector.memset(wrm[:], 0.0)
            for st in range(NST):
                nc.vector.memset(vts[st][:, :, HD:HD + 1], 1.0)

            # ---- HAM pre-warm: dummy matmuls while input DMAs stream ----
            wacc = psAV.tile([128, 512], F32, tag="psAV")
            for _ in range(6):
                nc.tensor.matmul(
                    wacc[:], wrm[:, 0:128], wrm[:], start=True, stop=True
                )

            # ---- projection building blocks ----
            def qk_group(which, ft, sb):
                """QT/KT [128 feat, 512 q] accumulated over D chunks (in x
                arrival order), evicted to bf16 with guide-token add
                (+1/8 scale folded into Q)."""
                if which == "q":
                    w_half = wq_sb["A"] if ft < 2 else wq_sb["B"]
                    add_sb, scale, dst = qa, 0.125, cq[ft]
                else:
                    w_half = wk_sb["A"] if ft < 2 else wk_sb["B"]
                    add_sb, scale, dst = ka, 1.0, ck[ft]
                fo = (ft % 2) * 128
                acc = psP.tile([128, 512], F32, tag="psP")
                for i, kc in enumerate(KC_ORDER):
                    yield lambda kc=kc, i=i, acc=acc: nc.tensor.matmul(
                        acc[:],
                        w_half[:, kc, fo:fo + 128],
                        xts[(kc // 2, sb)][:, kc % 2, :],
                        start=(i == 0),
                        stop=(i == NKC - 1),
                    )

                def evict(acc=acc):
                    nc.vector.tensor_scalar(
                        out=dst[:, sb * 512:(sb + 1) * 512],
                        in0=acc[:],
                        scalar1=scale,
                        scalar2=add_sb[:, ft:ft + 1],
                        op0=mybir.AluOpType.mult,
                        op1=mybir.AluOpType.add,
                    )
                    qk_done.add((which, ft, sb))

                yield evict

            v_done = [0]      # V groups fully emitted (gates AV emission)
            qk_done = set()   # (which, ft, sb) evictions emitted

            def v_group(st):
                """V [128 seq, 512 feat] natural layout, strided into vts."""
                acc = psP.tile([128, 512], F32, tag="psP")
                for i, kc in enumerate(KC_ORDER):
                    yield lambda kc=kc, i=i, acc=acc: nc.tensor.matmul(
                        acc[:],
                        xts[(kc // 2, st // 4)][:, kc % 2, (st % 4) * 128:(st % 4 + 1) * 128],
                        wv_sb[:, kc, :],
                        start=(i == 0),
                        stop=(i == NKC - 1),
                    )

                def evict(acc=acc):
                    nc.vector.tensor_copy(out=vts[st][:, :, 0:HD], in_=acc[:])
                    v_done[0] += 1

                yield evict

            def run(gen):
                for op in gen:
                    op()

            def filler_stream():
                yield from qk_group("q", 0, 1)      # Q01 (unit (0,1))
                yield from qk_group("q", 1, 0)
                yield from qk_group("k", 1, 0)
                yield from qk_group("k", 1, 1)
                yield from v_group(0)
                yield from v_group(1)
                yield from v_group(2)
                yield from v_group(3)
                yield from qk_group("q", 1, 1)
                for st in range(4, NST):
                    yield from v_group(st)
                yield from qk_group("q", 2, 0)
                yield from qk_group("k", 2, 0)
                yield from qk_group("k", 2, 1)
                yield from qk_group("q", 2, 1)
                yield from qk_group("q", 3, 0)
                yield from qk_group("k", 3, 0)
                yield from qk_group("k", 3, 1)
                yield from qk_group("q", 3, 1)

            # ---- output flush tracking ----
            done_units = set()

            def maybe_flush(hp, qb):
                done_units.add((hp, qb))
                if all((hp, q) in done_units for q in range(NQB)):
                    nc.sync.dma_start(out=yTd[hp], in_=yt[:, hp, :])

            # ---- AV + normalize ----
            av_tiles = {}   # u -> (av_even, av_odd)

            def av_ops(u, p, pairs):
                """AV accumulation MMs for kt pair p of unit u; on the last
                pair, the denominator/normalize chain + flush."""
                hp, qb = UNITS[u]
                if p == 0:
                    av_tiles[u] = (
                        psAV.tile([HD + 1, 512], F32, tag="psAV", name=f"av{u}e"),
                        psAV.tile([HD + 1, 512], F32, tag="psAV", name=f"av{u}o"),
                    )
                av_e, av_o = av_tiles[u]
                prA, prB = pairs[p]
                for j in range(2):
                    kt = 2 * p + j
                    nc.tensor.matmul(
                        av_e[:], vts[kt][:, 2 * hp, :], prA[:, j, :],
                        start=(kt == 0), stop=(kt == NST - 1),
                    )
                    nc.tensor.matmul(
                        av_o[:], vts[kt][:, 2 * hp + 1, :], prB[:, j, :],
                        start=(kt == 0), stop=(kt == NST - 1),
                    )
                if p == NPAIR - 1:
                    qsl = slice(qb * 512, (qb + 1) * 512)
                    for h_i, av in ((0, av_e), (1, av_o)):
                        den = norm_pool.tile([1, 512], F32, tag="den")
                        nc.vector.tensor_copy(out=den[:], in_=av[HD:HD + 1, :])
                        rec = norm_pool.tile([1, 512], F32, tag="rec")
                        nc.vector.reciprocal_approx_fast(out=rec[:], in_=den[:])
                        recb = norm_pool.tile([HD, 512], F32, tag="recb")
                        nc.gpsimd.partition_broadcast(recb[:], rec[:])
                        nc.vector.tensor_tensor(
                            out=yt[h_i * 64:h_i * 64 + 64, hp, qsl],
                            in0=av[0:HD, :],
                            in1=recb[:],
                            op=mybir.AluOpType.mult,
                        )
                    del av_tiles[u]
                    maybe_flush(hp, qb)

            # ---- phase 0: projections needed by unit (0,0) ----
            run(qk_group("q", 0, 0))
            run(qk_group("k", 0, 0))
            run(qk_group("k", 0, 1))

            UNITS = [(hp, qb) for hp in range(HPG // 2) for qb in range(NQB)]
            fillers = filler_stream()
            avq = []          # pending (u, p, pairs) AV groups, FIFO
            pairs_of = {}     # u -> list of (prA, prB) per pair

            def emit_pair(u, p):
                hp, qb = UNITS[u]
                qsl = slice(qb * 512, (qb + 1) * 512)
                scA = psA.tile([128, 2, 512], F32, tag="psA")
                scB = psA.tile([128, 2, 512], F32, tag="psA")
                for j in range(2):
                    kt = 2 * p + j
                    ksl = slice(kt * 128, (kt + 1) * 128)
                    nc.tensor.matmul(
                        scA[:, j, :], ck[hp][0:64, ksl], cq[hp][0:64, qsl],
                        start=True, stop=True,
                    )
                    nc.tensor.matmul(
                        scB[:, j, :], ck[hp][64:128, ksl], cq[hp][64:128, qsl],
                        start=True, stop=True,
                    )
                prA = probs_pool.tile([128, 2, 512], BF16, tag="probs")
                nc.scalar.activation(
                    out=prA[:], in_=scA[:],
                    func=mybir.ActivationFunctionType.Exp,
                )
                prB = probs_pool.tile([128, 2, 512], BF16, tag="probs")
                nc.scalar.activation(
                    out=prB[:], in_=scB[:],
                    func=mybir.ActivationFunctionType.Exp,
                )
                pairs_of[u].append((prA, prB))
                avq.append((u, p))

            def unit_ready(hp, qb):
                return (
                    ("q", hp, qb) in qk_done
                    and ("k", hp, 0) in qk_done
                    and ("k", hp, 1) in qk_done
                )

            for u in range(len(UNITS)):
                pairs_of[u] = []
                # correctness: this unit's projections must be in-stream
                # before its first score matmul is emitted
                while not unit_ready(*UNITS[u]):
                    op = next(fillers, None)
                    if op is None:
                        break
                    op()
                for p in range(NPAIR):
                    emit_pair(u, p)
                    # trailing AV groups: keep a reserve of AV work queued so
                    # the PE stream never runs dry late in the kernel; never
                    # pop before the needed V evictions are in-stream
                    pops = 0
                    max_pops = 2 if len(avq) > 10 else (1 if len(avq) > 7 else 0)
                    while avq and pops < max_pops:
                        au, ap_ = avq[0]
                        if v_done[0] < 2 * ap_ + 2:
                            break
                        avq.pop(0)
                        av_ops(au, ap_, pairs_of[au])
                        pops += 1
                    nf = FILLERS_PER_PAIR + (4 if pops == 0 else 0)
                    for _ in range(nf):
                        op = next(fillers, None)
                        if op is not None:
                            op()

            # drain: remaining fillers, then trailing AV groups
            for op in fillers:
                op()
            while avq:
                au, ap_ = avq.pop(0)
                av_ops(au, ap_, pairs_of[au])

    nc.finalize()
    return nc


def _get_nc():
    if "nc" not in _CACHE:
        _CACHE["nc"] = _build()
    return _CACHE["nc"]


def kernel(x, tokens, Wq, bq, Wk, bk, Wv, bv):
    x = np.asarray(x, dtype=np.float32)
    tokens = np.asarray(tokens, dtype=np.float32)
    Wq = np.asarray(Wq, dtype=np.float32)
    Wk = np.asarray(Wk, dtype=np.float32)
    Wv = np.asarray(Wv, dtype=np.float32)
    bq = np.asarray(bq, dtype=np.float32)
    bk = np.asarray(bk, dtype=np.float32)
    bv = np.asarray(bv, dtype=np.float32)

    bf16 = ml_dtypes.bfloat16
    in_maps = []
    for c in range(NCORES):
        b, g = divmod(c, 2)
        rows = slice(g * FPG, (g + 1) * FPG)
        tq = tokens[b, 0] @ Wq[rows].T + 2.0 * bq[rows]   # [512]
        tk = tokens[b, 0] @ Wk[rows].T + 2.0 * bk[rows]

        def packw(aT):
            # [D, C] -> [128, NKC, C] partition-major
            return np.ascontiguousarray(
                aT.reshape(NKC, 128, aT.shape[1]).transpose(1, 0, 2)
            ).astype(bf16)

        xTb = x[b].T.reshape(NKC, 128, S)   # [kc, p, s]
        wqT = Wq[rows].T
        wkT = Wk[rows].T
        m = {
            "wqA": packw(wqT[:, 0:256]),
            "wqB": packw(wqT[:, 256:512]),
            "wkA": packw(wkT[:, 0:256]),
            "wkB": packw(wkT[:, 256:512]),
            "wv": packw(Wv[rows].T),
            "qadd": np.ascontiguousarray((tq / 8.0).reshape(NFT, 128).T).astype(np.float32),
            "kadd": np.ascontiguousarray(tk.reshape(NFT, 128).T).astype(np.float32),
        }
        for ci, cl in enumerate("ABCD"):
            xp = xTb[2 * ci:2 * ci + 2].transpose(1, 0, 2)  # [128, 2, 1024]
            m[f"x{cl}0"] = np.ascontiguousarray(xp[:, :, 0:512]).astype(bf16)
            m[f"x{cl}1"] = np.ascontiguousarray(xp[:, :, 512:1024]).astype(bf16)
        in_maps.append(m)

    nc = _get_nc()
    trace = bool(int(os.environ.get("KERNEL_TRACE", "0")))
    res = run_bass_kernel_spmd(nc, in_maps, core_ids=list(range(NCORES)), trace=trace)
    if trace:
        _CACHE["last_results"] = res

    y = np.empty((B, S, D), dtype=np.float32)
    for c in range(NCORES):
        b, g = divmod(c, 2)
        yT = np.asarray(res.results[c]["yT"], dtype=np.float32)  # [4, 128, 1024]
        y[b, :, g * FPG:(g + 1) * FPG] = yT.reshape(FPG, S).T
    y += bv[None, None, :]
    return y


# revision 12
# speedup vs baseline: 1.0472x; 1.0472x over previous
"""Guide-token attention kernel for Trainium2 (8 NeuronCores).

Module: y[b] = softmax(((Q+tQ) @ (K+tK)^T)/sqrt(hd)) @ V  per head, where
  Q = x @ Wq^T + bq, K = x @ Wk^T + bk, V = x @ Wv^T + bv,
  tQ/tK are projections of a per-batch guide token (broadcast over seq).

Shapes: x [4, 1024, 1024], tokens [4, 1, 1024], W* [1024, 1024], b* [1024].
H=16 heads, hd=64.

Sharding: 8 cores = 4 batches x 2 head-groups (8 heads each); weights
column-sharded per head group; each core sees one batch -> no cross-core
communication.

Design notes (v2 — schedule rework of the 149us baseline):
  - PE is the bottleneck: ~384 N=512 matmul slots x ~220 ns ≈ 86 us
    (192 proj + 64 row-tiled score pairs + 128 AV).  Everything else
    (ACT exp 71 us, DVE ~40 us) must hide under the PE stream.
  - Input DMA split across BOTH hardware DGE queues (sync + scalar)
    with chunked SBUF tiles so the first projection matmuls start ~3 us
    in, and x/weight chunks unblock matmuls incrementally.
  - One unified PE program: per score pair (4 MMs, 2 exps) we interleave
    ~5 projection-filler MMs and up to 2 trailing AV groups, keeping PE
    dense and rate-matched against ACT while AV trails scores by >=2
    pairs (probs guaranteed ready -> no in-order stalls).
  - Normalize: reciprocal straight off the PSUM denominator row (no den
    copy), GpSimd partition-broadcast, one DVE multiply -> bf16 yT.
  - Output DMAd per 128-feature tile as soon as its 4 normalizes land.
"""

import os

import numpy as np
import ml_dtypes

import concourse.bass as bass
import concourse.tile as tile
from concourse import bacc
from concourse import mybir
from concourse.bass_utils import run_bass_kernel_spmd

B = 4
S = 1024
D = 1024
H = 16
HD = 64
NCORES = 8
FPG = 512          # features per head-group (8 heads * 64)
NKC = D // 128     # contraction chunks for projections
NFT = FPG // 128   # feature tiles per group
NST = S // 128     # sequence tiles
NQB = S // 512     # 512-wide query blocks
HPG = 8            # heads per group
NPAIR = NST // 2   # kt pairs per unit

BF16 = mybir.dt.bfloat16
F32 = mybir.dt.float32

# x chunk arrival order (see DMA queue layout below)
KC_ORDER = [0, 1, 4, 5, 2, 3, 6, 7]
FILLERS_PER_PAIR = 5

_CACHE = {}


def _build():
    nc = bacc.Bacc()

    # ---- DRAM inputs, chunked for DMA-queue parallelism + early unblock ----
    # sync queue:   x01, x23, wkA, qadd, kadd, wv   (+ output yT later)
    # scalar queue: wqA, x45, x67, wqB, wkB
    xd = {}
    for cname in ("xA0", "xB0", "xC0", "xD0", "xA1", "xB1", "xC1", "xD1"):
        xd[cname] = nc.declare_dram_parameter(cname, [128, 2, 512], BF16, isOutput=False)
    wqA = nc.declare_dram_parameter("wqA", [128, NKC, 256], BF16, isOutput=False)
    wqB = nc.declare_dram_parameter("wqB", [128, NKC, 256], BF16, isOutput=False)
    wkA = nc.declare_dram_parameter("wkA", [128, NKC, 256], BF16, isOutput=False)
    wkB = nc.declare_dram_parameter("wkB", [128, NKC, 256], BF16, isOutput=False)
    wvd = nc.declare_dram_parameter("wv", [128, NKC, FPG], BF16, isOutput=False)
    qaddd = nc.declare_dram_parameter("qadd", [128, NFT], F32, isOutput=False)
    kaddd = nc.declare_dram_parameter("kadd", [128, NFT], F32, isOutput=False)
    yTd = nc.declare_dram_parameter("yT", [NFT, 128, S], BF16, isOutput=True)

    with tile.TileContext(nc) as tc:
        with (
            tc.tile_pool(name="persist", bufs=1) as persist,
            tc.tile_pool(name="probs", bufs=36) as probs_pool,
            tc.tile_pool(name="norm", bufs=4) as norm_pool,
            tc.tile_pool(name="psP", bufs=2, space=bass.MemorySpace.PSUM) as psP,
            tc.tile_pool(name="psA", bufs=2, space=bass.MemorySpace.PSUM) as psA,
            tc.tile_pool(name="psAV", bufs=2, space=bass.MemorySpace.PSUM) as psAV,
        ):
            # ---- persistent SBUF tensors (chunked to match DMA granularity) ----
            xts = {(c, h): persist.tile([128, 2, 512], BF16, name=f"xt{c}{h}")
                   for c in range(4) for h in range(2)}
            wq_sb = {"A": persist.tile([128, NKC, 256], BF16, name="wq_a"),
                     "B": persist.tile([128, NKC, 256], BF16, name="wq_b")}
            wk_sb = {"A": persist.tile([128, NKC, 256], BF16, name="wk_a"),
                     "B": persist.tile([128, NKC, 256], BF16, name="wk_b")}
            wv_sb = persist.tile([128, NKC, FPG], BF16)
            qa = persist.tile([128, NFT], F32)
            ka = persist.tile([128, NFT], F32)
            cq = [persist.tile([128, S], BF16, name=f"cq{i}") for i in range(NFT)]   # cQT/8
            ck = [persist.tile([128, S], BF16, name=f"ck{i}") for i in range(NFT)]   # cKT
            vts = [persist.tile([128, HPG, HD + 1], BF16, name=f"vt{i}") for i in range(NST)]
            yt = persist.tile([128, NFT, S], BF16)
            wrm = persist.tile([128, 512], BF16)

            # ---- input DMAs on both HW DGE queues ----
            nc.sync.dma_start(out=xts[(0, 0)][:], in_=xd["xA0"][:])
            nc.sync.dma_start(out=xts[(1, 0)][:], in_=xd["xB0"][:])
            nc.sync.dma_start(out=qa[:], in_=qaddd[:])
            nc.sync.dma_start(out=ka[:], in_=kaddd[:])
            nc.sync.dma_start(out=wk_sb["A"][:], in_=wkA[:])
            nc.sync.dma_start(out=xts[(0, 1)][:], in_=xd["xA1"][:])
            nc.sync.dma_start(out=xts[(1, 1)][:], in_=xd["xB1"][:])
            nc.scalar.dma_start(out=wq_sb["A"][:], in_=wqA[:])
            nc.scalar.dma_start(out=xts[(2, 0)][:], in_=xd["xC0"][:])
            nc.scalar.dma_start(out=xts[(3, 0)][:], in_=xd["xD0"][:])
            nc.scalar.dma_start(out=xts[(2, 1)][:], in_=xd["xC1"][:])
            nc.scalar.dma_start(out=xts[(3, 1)][:], in_=xd["xD1"][:])
            nc.scalar.dma_start(out=wv_sb[:], in_=wvd[:])
            nc.scalar.dma_start(out=wq_sb["B"][:], in_=wqB[:])
            nc.scalar.dma_start(out=wk_sb["B"][:], in_=wkB[:])

            # ones columns for the AV denominator rows + HAM warmup source
            nc.vector.memset(wrm[:], 0.0)
            for st in range(NST):
                nc.vector.memset(vts[st][:, :, HD:HD + 1], 1.0)

            # ---- HAM pre-warm: dummy matmuls while input DMAs stream ----
            wacc = psAV.tile([128, 512], F32, tag="psAV")
            for _ in range(6):
                nc.tensor.matmul(
                    wacc[:], wrm[:, 0:128], wrm[:], start=True, stop=True
                )

            # ---- projection building blocks ----
            def qk_group(which, ft, sb):
                """QT/KT [128 feat, 512 q] accumulated over D chunks (in x
                arrival order), evicted to bf16 with guide-token add
                (+1/8 scale folded into Q)."""
                if which == "q":
                    w_half = wq_sb["A"] if ft < 2 else wq_sb["B"]
                    add_sb, scale, dst = qa, 0.125, cq[ft]
                else:
                    w_half = wk_sb["A"] if ft < 2 else wk_sb["B"]
                    add_sb, scale, dst = ka, 1.0, ck[ft]
                fo = (ft % 2) * 128
                acc = psP.tile([128, 512], F32, tag="psP")
                for i, kc in enumerate(KC_ORDER):
                    yield lambda kc=kc, i=i, acc=acc: nc.tensor.matmul(
                        acc[:],
                        w_half[:, kc, fo:fo + 128],
                        xts[(kc // 2, sb)][:, kc % 2, :],
                        start=(i == 0),
                        stop=(i == NKC - 1),
                    )

                def evict(acc=acc):
                    nc.vector.tensor_scalar(
                        out=dst[:, sb * 512:(sb + 1) * 512],
                        in0=acc[:],
                        scalar1=scale,
                        scalar2=add_sb[:, ft:ft + 1],
                        op0=mybir.AluOpType.mult,
                        op1=mybir.AluOpType.add,
                    )
                    qk_done.add((which, ft, sb))

                yield evict

            v_done = [0]      # V groups fully emitted (gates AV emission)
            qk_done = set()   # (which, ft, sb) evictions emitted

            def v_group(st):
                """V [128 seq, 512 feat] natural layout, strided into vts."""
                acc = psP.tile([128, 512], F32, tag="psP")
                for i, kc in enumerate(KC_ORDER):
                    yield lambda kc=kc, i=i, acc=acc: nc.tensor.matmul(
                        acc[:],
                        xts[(kc // 2, st // 4)][:, kc % 2, (st % 4) * 128:(st % 4 + 1) * 128],
                        wv_sb[:, kc, :],
                        start=(i == 0),
                        stop=(i == NKC - 1),
                    )

                def evict(acc=acc):
                    nc.vector.tensor_copy(out=vts[st][:, :, 0:HD], in_=acc[:])
                    v_done[0] += 1

                yield evict

            def run(gen):
                for op in gen:
                    op()

            def filler_stream():
                yield from qk_group("q", 0, 1)      # Q01 (unit (0,1))
                yield from qk_group("q", 1, 0)
                yield from qk_group("k", 1, 0)
                yield from qk_group("k", 1, 1)
                yield from v_group(0)
                yield from v_group(1)
                yield from v_group(2)
                yield from v_group(3)
                yield from qk_group("q", 1, 1)
                for st in range(4, NST):
                    yield from v_group(st)
                yield from qk_group("q", 2, 0)
                yield from qk_group("k", 2, 0)
                yield from qk_group("k", 2, 1)
                yield from qk_group("q", 2, 1)
                yield from qk_group("q", 3, 0)
                yield from qk_group("k", 3, 0)
                yield from qk_group("k", 3, 1)
                yield from qk_group("q", 3, 1)

            # ---- output flush tracking ----
            done_units = set()

            def maybe_flush(hp, qb):
                done_units.add((hp, qb))
                if all((hp, q) in done_units for q in range(NQB)):
                    nc.sync.dma_start(out=yTd[hp], in_=yt[:, hp, :])

            # ---- AV + normalize ----
            av_tiles = {}   # u -> (av_even, av_odd)

            def av_ops(u, p, pairs):
                """AV accumulation MMs for kt pair p of unit u; on the last
                pair, the denominator/normalize chain + flush."""
                hp, qb = UNITS[u]
                if p == 0:
                    av_tiles[u] = (
                        psAV.tile([HD + 1, 512], F32, tag="psAV", name=f"av{u}e"),
                        psAV.tile([HD + 1, 512], F32, tag="psAV", name=f"av{u}o"),
                    )
                av_e, av_o = av_tiles[u]
                prA, prB = pairs[p]
                for j in range(2):
                    kt = 2 * p + j
                    nc.tensor.matmul(
                        av_e[:], vts[kt][:, 2 * hp, :], prA[:, j, :],
                        start=(kt == 0), stop=(kt == NST - 1),
                    )
                    nc.tensor.matmul(
                        av_o[:], vts[kt][:, 2 * hp + 1, :], prB[:, j, :],
                        start=(kt == 0), stop=(kt == NST - 1),
                    )
                if p == NPAIR - 1:
                    qsl = slice(qb * 512, (qb + 1) * 512)
                    for h_i, av in ((0, av_e), (1, av_o)):
                        den = norm_pool.tile([1, 512], F32, tag="den")
                        nc.vector.tensor_copy(out=den[:], in_=av[HD:HD + 1, :])
                        rec = norm_pool.tile([1, 512], F32, tag="rec")
                        nc.vector.reciprocal_approx_fast(out=rec[:], in_=den[:])
                        recb = norm_pool.tile([HD, 512], F32, tag="recb")
                        nc.gpsimd.partition_broadcast(recb[:], rec[:])
                        nc.vector.tensor_tensor(
                            out=yt[h_i * 64:h_i * 64 + 64, hp, qsl],
                            in0=av[0:HD, :],
                            in1=recb[:],
                            op=mybir.AluOpType.mult,
                        )
                    del av_tiles[u]
                    maybe_flush(hp, qb)

            # ---- phase 0: projections needed by unit (0,0) ----
            run(qk_group("q", 0, 0))
            run(qk_group("k", 0, 0))
            run(qk_group("k", 0, 1))

            UNITS = [(hp, qb) for hp in range(HPG // 2) for qb in range(NQB)]
            fillers = filler_stream()
            avq = []          # pending (u, p, pairs) AV groups, FIFO
            pairs_of = {}     # u -> list of (prA, prB) per pair

            def emit_pair(u, p):
                hp, qb = UNITS[u]
                qsl = slice(qb * 512, (qb + 1) * 512)
                scA = psA.tile([128, 2, 512], F32, tag="psA")
                scB = psA.tile([128, 2, 512], F32, tag="psA")
                for j in range(2):
                    kt = 2 * p + j
                    ksl = slice(kt * 128, (kt + 1) * 128)
                    nc.tensor.matmul(
                        scA[:, j, :], ck[hp][0:64, ksl], cq[hp][0:64, qsl],
                        start=True, stop=True,
                    )
                    nc.tensor.matmul(
                        scB[:, j, :], ck[hp][64:128, ksl], cq[hp][64:128, qsl],
                        start=True, stop=True,
                    )
                prA = probs_pool.tile([128, 2, 512], BF16, tag="probs")
                nc.scalar.activation(
                    out=prA[:], in_=scA[:],
                    func=mybir.ActivationFunctionType.Exp,
                )
                prB = probs_pool.tile([128, 2, 512], BF16, tag="probs")
                nc.scalar.activation(
                    out=prB[:], in_=scB[:],
                    func=mybir.ActivationFunctionType.Exp,
                )
                pairs_of[u].append((prA, prB))
                avq.append((u, p))

            def unit_ready(hp, qb):
                return (
                    ("q", hp, qb) in qk_done
                    and ("k", hp, 0) in qk_done
                    and ("k", hp, 1) in qk_done
                )

            for u in range(len(UNITS)):
                pairs_of[u] = []
                # correctness: this unit's projections must be in-stream
                # before its first score matmul is emitted
                while not unit_ready(*UNITS[u]):
                    op = next(fillers, None)
                    if op is None:
                        break
                    op()
                for p in range(NPAIR):
                    emit_pair(u, p)
                    # trailing AV groups: keep a reserve of AV work queued so
                    # the PE stream never runs dry late in the kernel; never
                    # pop before the needed V evictions are in-stream
                    pops = 0
                    if u >= 6:
                        max_pops = 2 if len(avq) > 2 else 1
                    else:
                        max_pops = 2 if len(avq) > 10 else (1 if len(avq) > 7 else 0)
                    while avq and pops < max_pops:
                        au, ap_ = avq[0]
                        if v_done[0] < 2 * ap_ + 2:
                            break
                        avq.pop(0)
                        av_ops(au, ap_, pairs_of[au])
                        pops += 1
                    nf = FILLERS_PER_PAIR + (4 if pops == 0 else 0)
                    for _ in range(nf):
                        op = next(fillers, None)
                        if op is not None:
                            op()

            # drain: remaining fillers, then trailing AV groups
            for op in fillers:
                op()
            while avq:
                au, ap_ = avq.pop(0)
                av_ops(au, ap_, pairs_of[au])

    nc.finalize()
    return nc


def _get_nc():
    if "nc" not in _CACHE:
        _CACHE["nc"] = _build()
    return _CACHE["nc"]


def kernel(x, tokens, Wq, bq, Wk, bk, Wv, bv):
    x = np.asarray(x, dtype=np.float32)
    tokens = np.asarray(tokens, dtype=np.float32)
    Wq = np.asarray(Wq, dtype=np.float32)
    Wk = np.asarray(Wk, dtype=np.float32)
    Wv = np.asarray(Wv, dtype=np.float32)
    bq = np.asarray(bq, dtype=np.float32)
    bk = np.asarray(bk, dtype=np.float32)
    bv = np.asarray(bv, dtype=np.float32)

    bf16 = ml_dtypes.bfloat16
    in_maps = []
    for c in range(NCORES):
        b, g = divmod(c, 2)
        rows = slice(g * FPG, (g + 1) * FPG)
        tq = tokens[b, 0] @ Wq[rows].T + 2.0 * bq[rows]   # [512]
        tk = tokens[b, 0] @ Wk[rows].T + 2.0 * bk[rows]

        def packw(aT):
            # [D, C] -> [128, NKC, C] partition-major
            return np.ascontiguousarray(
                aT.reshape(NKC, 128, aT.shape[1]).transpose(1, 0, 2)
            ).astype(bf16)

        xTb = x[b].T.reshape(NKC, 128, S)   # [kc, p, s]
        wqT = Wq[rows].T
        wkT = Wk[rows].T
        m = {
            "wqA": packw(wqT[:, 0:256]),
            "wqB": packw(wqT[:, 256:512]),
            "wkA": packw(wkT[:, 0:256]),
            "wkB": packw(wkT[:, 256:512]),
            "wv": packw(Wv[rows].T),
            "qadd": np.ascontiguousarray((tq / 8.0).reshape(NFT, 128).T).astype(np.float32),
            "kadd": np.ascontiguousarray(tk.reshape(NFT, 128).T).astype(np.float32),
        }
        for ci, cl in enumerate("ABCD"):
            xp = xTb[2 * ci:2 * ci + 2].transpose(1, 0, 2)  # [128, 2, 1024]
            m[f"x{cl}0"] = np.ascontiguousarray(xp[:, :, 0:512]).astype(bf16)
            m[f"x{cl}1"] = np.ascontiguousarray(xp[:, :, 512:1024]).astype(bf16)
        in_maps.append(m)

    nc = _get_nc()
    trace = bool(int(os.environ.get("KERNEL_TRACE", "0")))
    res = run_bass_kernel_spmd(nc, in_maps, core_ids=list(range(NCORES)), trace=trace)
    if trace:
        _CACHE["last_results"] = res

    y = np.empty((B, S, D), dtype=np.float32)
    for c in range(NCORES):
        b, g = divmod(c, 2)
        yT = np.asarray(res.results[c]["yT"], dtype=np.float32)  # [4, 128, 1024]
        y[b, :, g * FPG:(g + 1) * FPG] = yT.reshape(FPG, S).T
    y += bv[None, None, :]
    return y


# revision 13
# speedup vs baseline: 1.0498x; 1.0024x over previous
"""Guide-token attention kernel for Trainium2 (8 NeuronCores).

Module: y[b] = softmax(((Q+tQ) @ (K+tK)^T)/sqrt(hd)) @ V  per head, where
  Q = x @ Wq^T + bq, K = x @ Wk^T + bk, V = x @ Wv^T + bv,
  tQ/tK are projections of a per-batch guide token (broadcast over seq).

Shapes: x [4, 1024, 1024], tokens [4, 1, 1024], W* [1024, 1024], b* [1024].
H=16 heads, hd=64.

Sharding: 8 cores = 4 batches x 2 head-groups (8 heads each); weights
column-sharded per head group; each core sees one batch -> no cross-core
communication.

Design notes (v2 — schedule rework of the 149us baseline):
  - PE is the bottleneck: ~384 N=512 matmul slots x ~220 ns ≈ 86 us
    (192 proj + 64 row-tiled score pairs + 128 AV).  Everything else
    (ACT exp 71 us, DVE ~40 us) must hide under the PE stream.
  - Input DMA split across BOTH hardware DGE queues (sync + scalar)
    with chunked SBUF tiles so the first projection matmuls start ~3 us
    in, and x/weight chunks unblock matmuls incrementally.
  - One unified PE program: per score pair (4 MMs, 2 exps) we interleave
    ~5 projection-filler MMs and up to 2 trailing AV groups, keeping PE
    dense and rate-matched against ACT while AV trails scores by >=2
    pairs (probs guaranteed ready -> no in-order stalls).
  - Normalize: reciprocal straight off the PSUM denominator row (no den
    copy), GpSimd partition-broadcast, one DVE multiply -> bf16 yT.
  - Output DMAd per 128-feature tile as soon as its 4 normalizes land.
"""

import os

import numpy as np
import ml_dtypes

import concourse.bass as bass
import concourse.tile as tile
from concourse import bacc
from concourse import mybir
from concourse.bass_utils import run_bass_kernel_spmd

B = 4
S = 1024
D = 1024
H = 16
HD = 64
NCORES = 8
FPG = 512          # features per head-group (8 heads * 64)
NKC = D // 128     # contraction chunks for projections
NFT = FPG // 128   # feature tiles per group
NST = S // 128     # sequence tiles
NQB = S // 512     # 512-wide query blocks
HPG = 8            # heads per group
NPAIR = NST // 2   # kt pairs per unit

BF16 = mybir.dt.bfloat16
F32 = mybir.dt.float32

# x chunk arrival order (see DMA queue layout below)
KC_ORDER = [0, 1, 2, 3, 4, 5, 6, 7]
FILLERS_PER_PAIR = 5

_CACHE = {}


def _build():
    nc = bacc.Bacc()

    # ---- DRAM inputs, chunked for DMA-queue parallelism + early unblock ----
    # sync queue:   x01, x23, wkA, qadd, kadd, wv   (+ output yT later)
    # scalar queue: wqA, x45, x67, wqB, wkB
    xd = {}
    for cname in ("xA0", "xB0", "xC0", "xD0", "xA1", "xB1", "xC1", "xD1"):
        xd[cname] = nc.declare_dram_parameter(cname, [128, 2, 512], BF16, isOutput=False)
    wqA = nc.declare_dram_parameter("wqA", [128, NKC, 256], BF16, isOutput=False)
    wqB = nc.declare_dram_parameter("wqB", [128, NKC, 256], BF16, isOutput=False)
    wkA = nc.declare_dram_parameter("wkA", [128, NKC, 256], BF16, isOutput=False)
    wkB = nc.declare_dram_parameter("wkB", [128, NKC, 256], BF16, isOutput=False)
    wvd = nc.declare_dram_parameter("wv", [128, NKC, FPG], BF16, isOutput=False)
    qaddd = nc.declare_dram_parameter("qadd", [128, NFT], F32, isOutput=False)
    kaddd = nc.declare_dram_parameter("kadd", [128, NFT], F32, isOutput=False)
    yTd = nc.declare_dram_parameter("yT", [NFT, 128, S], BF16, isOutput=True)

    with tile.TileContext(nc) as tc:
        with (
            tc.tile_pool(name="persist", bufs=1) as persist,
            tc.tile_pool(name="probs", bufs=36) as probs_pool,
            tc.tile_pool(name="norm", bufs=4) as norm_pool,
            tc.tile_pool(name="psP", bufs=2, space=bass.MemorySpace.PSUM) as psP,
            tc.tile_pool(name="psA", bufs=2, space=bass.MemorySpace.PSUM) as psA,
            tc.tile_pool(name="psAV", bufs=2, space=bass.MemorySpace.PSUM) as psAV,
        ):
            # ---- persistent SBUF tensors (chunked to match DMA granularity) ----
            xts = {(c, h): persist.tile([128, 2, 512], BF16, name=f"xt{c}{h}")
                   for c in range(4) for h in range(2)}
            wq_sb = {"A": persist.tile([128, NKC, 256], BF16, name="wq_a"),
                     "B": persist.tile([128, NKC, 256], BF16, name="wq_b")}
            wk_sb = {"A": persist.tile([128, NKC, 256], BF16, name="wk_a"),
                     "B": persist.tile([128, NKC, 256], BF16, name="wk_b")}
            wv_sb = persist.tile([128, NKC, FPG], BF16)
            qa = persist.tile([128, NFT], F32)
            ka = persist.tile([128, NFT], F32)
            cq = [persist.tile([128, S], BF16, name=f"cq{i}") for i in range(NFT)]   # cQT/8
            ck = [persist.tile([128, S], BF16, name=f"ck{i}") for i in range(NFT)]   # cKT
            vts = [persist.tile([128, HPG, HD + 1], BF16, name=f"vt{i}") for i in range(NST)]
            yt = persist.tile([128, NFT, S], BF16)
            wrm = persist.tile([128, 512], BF16)

            # ---- input DMAs on both HW DGE queues ----
            nc.sync.dma_start(out=xts[(0, 0)][:], in_=xd["xA0"][:])
            nc.sync.dma_start(out=xts[(1, 0)][:], in_=xd["xB0"][:])
            nc.sync.dma_start(out=qa[:], in_=qaddd[:])
            nc.sync.dma_start(out=ka[:], in_=kaddd[:])
            nc.sync.dma_start(out=wk_sb["A"][:], in_=wkA[:])
            nc.sync.dma_start(out=xts[(0, 1)][:], in_=xd["xA1"][:])
            nc.sync.dma_start(out=xts[(1, 1)][:], in_=xd["xB1"][:])
            nc.scalar.dma_start(out=wq_sb["A"][:], in_=wqA[:])
            nc.scalar.dma_start(out=xts[(2, 0)][:], in_=xd["xC0"][:])
            nc.scalar.dma_start(out=xts[(3, 0)][:], in_=xd["xD0"][:])
            nc.scalar.dma_start(out=xts[(2, 1)][:], in_=xd["xC1"][:])
            nc.scalar.dma_start(out=xts[(3, 1)][:], in_=xd["xD1"][:])
            nc.scalar.dma_start(out=wv_sb[:], in_=wvd[:])
            nc.scalar.dma_start(out=wq_sb["B"][:], in_=wqB[:])
            nc.scalar.dma_start(out=wk_sb["B"][:], in_=wkB[:])

            # ones columns for the AV denominator rows + HAM warmup source
            nc.vector.memset(wrm[:], 0.0)
            for st in range(NST):
                nc.vector.memset(vts[st][:, :, HD:HD + 1], 1.0)

            # ---- HAM pre-warm: dummy matmuls while input DMAs stream ----
            wacc = psAV.tile([128, 512], F32, tag="psAV")
            for _ in range(6):
                nc.tensor.matmul(
                    wacc[:], wrm[:, 0:128], wrm[:], start=True, stop=True
                )

            # ---- projection building blocks ----
            def qk_group(which, ft, sb):
                """QT/KT [128 feat, 512 q] accumulated over D chunks (in x
                arrival order), evicted to bf16 with guide-token add
                (+1/8 scale folded into Q)."""
                if which == "q":
                    w_half = wq_sb["A"] if ft < 2 else wq_sb["B"]
                    add_sb, scale, dst = qa, 0.125, cq[ft]
                else:
                    w_half = wk_sb["A"] if ft < 2 else wk_sb["B"]
                    add_sb, scale, dst = ka, 1.0, ck[ft]
                fo = (ft % 2) * 128
                acc = psP.tile([128, 512], F32, tag="psP")
                for i, kc in enumerate(KC_ORDER):
                    yield lambda kc=kc, i=i, acc=acc: nc.tensor.matmul(
                        acc[:],
                        w_half[:, kc, fo:fo + 128],
                        xts[(kc // 2, sb)][:, kc % 2, :],
                        start=(i == 0),
                        stop=(i == NKC - 1),
                    )

                def evict(acc=acc):
                    nc.vector.tensor_scalar(
                        out=dst[:, sb * 512:(sb + 1) * 512],
                        in0=acc[:],
                        scalar1=scale,
                        scalar2=add_sb[:, ft:ft + 1],
                        op0=mybir.AluOpType.mult,
                        op1=mybir.AluOpType.add,
                    )
                    qk_done.add((which, ft, sb))

                yield evict

            v_done = [0]      # V groups fully emitted (gates AV emission)
            qk_done = set()   # (which, ft, sb) evictions emitted

            def v_group(st):
                """V [128 seq, 512 feat] natural layout, strided into vts."""
                acc = psP.tile([128, 512], F32, tag="psP")
                for i, kc in enumerate(KC_ORDER):
                    yield lambda kc=kc, i=i, acc=acc: nc.tensor.matmul(
                        acc[:],
                        xts[(kc // 2, st // 4)][:, kc % 2, (st % 4) * 128:(st % 4 + 1) * 128],
                        wv_sb[:, kc, :],
                        start=(i == 0),
                        stop=(i == NKC - 1),
                    )

                def evict(acc=acc):
                    nc.vector.tensor_copy(out=vts[st][:, :, 0:HD], in_=acc[:])
                    v_done[0] += 1

                yield evict

            def run(gen):
                for op in gen:
                    op()

            def filler_stream():
                yield from qk_group("k", 0, 1)      # keys sb1 (unit0 p2/p3)
                yield from qk_group("q", 0, 1)      # Q01 (unit (0,1))
                yield from qk_group("q", 1, 0)
                yield from qk_group("k", 1, 0)
                yield from qk_group("k", 1, 1)
                yield from v_group(0)
                yield from v_group(1)
                yield from v_group(2)
                yield from v_group(3)
                yield from qk_group("q", 1, 1)
                for st in range(4, NST):
                    yield from v_group(st)
                yield from qk_group("q", 2, 0)
                yield from qk_group("k", 2, 0)
                yield from qk_group("k", 2, 1)
                yield from qk_group("q", 2, 1)
                yield from qk_group("q", 3, 0)
                yield from qk_group("k", 3, 0)
                yield from qk_group("k", 3, 1)
                yield from qk_group("q", 3, 1)

            # ---- output flush tracking ----
            done_units = set()

            def maybe_flush(hp, qb):
                done_units.add((hp, qb))
                if all((hp, q) in done_units for q in range(NQB)):
                    nc.sync.dma_start(out=yTd[hp], in_=yt[:, hp, :])

            # ---- AV + normalize ----
            av_tiles = {}   # u -> (av_even, av_odd)

            def av_ops(u, p, pairs):
                """AV accumulation MMs for kt pair p of unit u; on the last
                pair, the denominator/normalize chain + flush."""
                hp, qb = UNITS[u]
                if p == 0:
                    av_tiles[u] = (
                        psAV.tile([HD + 1, 512], F32, tag="psAV", name=f"av{u}e"),
                        psAV.tile([HD + 1, 512], F32, tag="psAV", name=f"av{u}o"),
                    )
                av_e, av_o = av_tiles[u]
                prA, prB = pairs[p]
                for j in range(2):
                    kt = 2 * p + j
                    nc.tensor.matmul(
                        av_e[:], vts[kt][:, 2 * hp, :], prA[:, j, :],
                        start=(kt == 0), stop=(kt == NST - 1),
                    )
                    nc.tensor.matmul(
                        av_o[:], vts[kt][:, 2 * hp + 1, :], prB[:, j, :],
                        start=(kt == 0), stop=(kt == NST - 1),
                    )
                if p == NPAIR - 1:
                    qsl = slice(qb * 512, (qb + 1) * 512)
                    for h_i, av in ((0, av_e), (1, av_o)):
                        den = norm_pool.tile([1, 512], F32, tag="den")
                        nc.vector.tensor_copy(out=den[:], in_=av[HD:HD + 1, :])
                        rec = norm_pool.tile([1, 512], F32, tag="rec")
                        nc.vector.reciprocal_approx_fast(out=rec[:], in_=den[:])
                        recb = norm_pool.tile([HD, 512], F32, tag="recb")
                        nc.gpsimd.partition_broadcast(recb[:], rec[:])
                        nc.vector.tensor_tensor(
                            out=yt[h_i * 64:h_i * 64 + 64, hp, qsl],
                            in0=av[0:HD, :],
                            in1=recb[:],
                            op=mybir.AluOpType.mult,
                        )
                    del av_tiles[u]
                    maybe_flush(hp, qb)

            # ---- phase 0: projections needed by unit (0,0) pair 0 ----
            run(qk_group("q", 0, 0))
            run(qk_group("k", 0, 0))

            UNITS = [(hp, qb) for hp in range(HPG // 2) for qb in range(NQB)]
            fillers = filler_stream()
            avq = []          # pending (u, p, pairs) AV groups, FIFO
            pairs_of = {}     # u -> list of (prA, prB) per pair

            def emit_pair(u, p):
                hp, qb = UNITS[u]
                qsl = slice(qb * 512, (qb + 1) * 512)
                scA = psA.tile([128, 2, 512], F32, tag="psA")
                scB = psA.tile([128, 2, 512], F32, tag="psA")
                for j in range(2):
                    kt = 2 * p + j
                    ksl = slice(kt * 128, (kt + 1) * 128)
                    nc.tensor.matmul(
                        scA[:, j, :], ck[hp][0:64, ksl], cq[hp][0:64, qsl],
                        start=True, stop=True,
                    )
                    nc.tensor.matmul(
                        scB[:, j, :], ck[hp][64:128, ksl], cq[hp][64:128, qsl],
                        start=True, stop=True,
                    )
                prA = probs_pool.tile([128, 2, 512], BF16, tag="probs")
                nc.scalar.activation(
                    out=prA[:], in_=scA[:],
                    func=mybir.ActivationFunctionType.Exp,
                )
                prB = probs_pool.tile([128, 2, 512], BF16, tag="probs")
                nc.scalar.activation(
                    out=prB[:], in_=scB[:],
                    func=mybir.ActivationFunctionType.Exp,
                )
                pairs_of[u].append((prA, prB))
                avq.append((u, p))

            def pair_ready(hp, qb, p):
                return ("q", hp, qb) in qk_done and ("k", hp, p // 2) in qk_done

            for u in range(len(UNITS)):
                pairs_of[u] = []
                for p in range(NPAIR):
                    # correctness: the projections this pair reads must be
                    # in-stream before its score matmuls are emitted
                    while not pair_ready(*UNITS[u], p):
                        op = next(fillers, None)
                        if op is None:
                            break
                        op()
                    emit_pair(u, p)
                    # trailing AV groups: keep a reserve of AV work queued so
                    # the PE stream never runs dry late in the kernel; never
                    # pop before the needed V evictions are in-stream
                    pops = 0
                    if u >= 6:
                        max_pops = 2 if len(avq) > 1 else 1
                    else:
                        max_pops = 2 if len(avq) > 10 else (1 if len(avq) > 8 else 0)
                    while avq and pops < max_pops:
                        au, ap_ = avq[0]
                        if v_done[0] < 2 * ap_ + 2:
                            break
                        avq.pop(0)
                        av_ops(au, ap_, pairs_of[au])
                        pops += 1
                    nf = FILLERS_PER_PAIR + (4 if pops == 0 else 0)
                    for _ in range(nf):
                        op = next(fillers, None)
                        if op is not None:
                            op()

            # drain: remaining fillers, then trailing AV groups
            for op in fillers:
                op()
            while avq:
                au, ap_ = avq.pop(0)
                av_ops(au, ap_, pairs_of[au])

    nc.finalize()
    return nc


def _get_nc():
    if "nc" not in _CACHE:
        _CACHE["nc"] = _build()
    return _CACHE["nc"]


def kernel(x, tokens, Wq, bq, Wk, bk, Wv, bv):
    x = np.asarray(x, dtype=np.float32)
    tokens = np.asarray(tokens, dtype=np.float32)
    Wq = np.asarray(Wq, dtype=np.float32)
    Wk = np.asarray(Wk, dtype=np.float32)
    Wv = np.asarray(Wv, dtype=np.float32)
    bq = np.asarray(bq, dtype=np.float32)
    bk = np.asarray(bk, dtype=np.float32)
    bv = np.asarray(bv, dtype=np.float32)

    bf16 = ml_dtypes.bfloat16
    in_maps = []
    for c in range(NCORES):
        b, g = divmod(c, 2)
        rows = slice(g * FPG, (g + 1) * FPG)
        tq = tokens[b, 0] @ Wq[rows].T + 2.0 * bq[rows]   # [512]
        tk = tokens[b, 0] @ Wk[rows].T + 2.0 * bk[rows]

        def packw(aT):
            # [D, C] -> [128, NKC, C] partition-major
            return np.ascontiguousarray(
                aT.reshape(NKC, 128, aT.shape[1]).transpose(1, 0, 2)
            ).astype(bf16)

        xTb = x[b].T.reshape(NKC, 128, S)   # [kc, p, s]
        wqT = Wq[rows].T
        wkT = Wk[rows].T
        m = {
            "wqA": packw(wqT[:, 0:256]),
            "wqB": packw(wqT[:, 256:512]),
            "wkA": packw(wkT[:, 0:256]),
            "wkB": packw(wkT[:, 256:512]),
            "wv": packw(Wv[rows].T),
            "qadd": np.ascontiguousarray((tq / 8.0).reshape(NFT, 128).T).astype(np.float32),
            "kadd": np.ascontiguousarray(tk.reshape(NFT, 128).T).astype(np.float32),
        }
        for ci, cl in enumerate("ABCD"):
            xp = xTb[2 * ci:2 * ci + 2].transpose(1, 0, 2)  # [128, 2, 1024]
            m[f"x{cl}0"] = np.ascontiguousarray(xp[:, :, 0:512]).astype(bf16)
            m[f"x{cl}1"] = np.ascontiguousarray(xp[:, :, 512:1024]).astype(bf16)
        in_maps.append(m)

    nc = _get_nc()
    trace = bool(int(os.environ.get("KERNEL_TRACE", "0")))
    res = run_bass_kernel_spmd(nc, in_maps, core_ids=list(range(NCORES)), trace=trace)
    if trace:
        _CACHE["last_results"] = res

    y = np.empty((B, S, D), dtype=np.float32)
    for c in range(NCORES):
        b, g = divmod(c, 2)
        yT = np.asarray(res.results[c]["yT"], dtype=np.float32)  # [4, 128, 1024]
        y[b, :, g * FPG:(g + 1) * FPG] = yT.reshape(FPG, S).T
    y += bv[None, None, :]
    return y
